# revision 7
# baseline (speedup 1.0000x reference)
"""Mixture-of-Experts forward on 8 Trainium2 NeuronCores.

Strategy: expert-parallel. The host dispatches tokens to experts (gather by
router indices), each of the 8 cores runs one expert's MLP
(Linear(D->H) + ReLU + Linear(H->D)) over its token batch in fp16 with fp32
PSUM accumulation, and the host applies router probabilities and
scatter-adds the per-(token, slot) outputs back to the full [N, D] output.

Device-side layout: tokens live on the matmul moving (free) dimension and
the contraction dims (D for layer 1, H for layer 2) live on SBUF
partitions, so neither weights nor activations ever need an on-device
transpose: layer 1 computes hT = W1slice.T @ xT and layer 2 computes
yT = W2slice.T @ hT.
"""

import json

import numpy as np

N_TOK = 8192
D, H, E, TOPK, P = 1024, 4096, 8, 2, 128
T = 512  # main token tile per matmul (moving dim; <=512 fp32 PSUM bank)
DO, HO = D // P, H // P


def _block_schedule(C):
    """Split capacity C (multiple of 128) into matmul moving-dim blocks."""
    blocks = [T] * (C // T)
    if C % T:
        blocks.append(C % T)
    return blocks

_NC_CACHE = {}


# --- walrus workaround -----------------------------------------------------
# The installed walrus rejects control instructions (Drain) carrying more
# than a couple of semaphore waits ("Too many sync wait commands",
# CoreV3GenImpl.cpp setupSyncWait<...CTRL_NO_STRUCT>). TileContext's final
# global-clock drain collects one wait per logical processor, which trips
# this. Split long on_wait lists across several consecutive same-engine
# Drain instructions at the BIR-JSON level.

def _split_waits(bir, max_waits=1):
    for fn in bir.get("functions", []):
        for blk in fn.get("blocks", []):
            new_insts = []
            for inst in blk.get("instructions", []):
                si = inst.get("sync_info") or {}
                waits = si.get("on_wait") or []
                if len(waits) <= max_waits:
                    new_insts.append(inst)
                    continue
                chunks = [
                    waits[i : i + max_waits]
                    for i in range(0, len(waits), max_waits)
                ]
                for ci, chunk in enumerate(chunks[:-1]):
                    new_insts.append(
                        {
                            "debug": inst.get("debug"),
                            "engine": inst["engine"],
                            "ins": [],
                            "name": f"{inst['name']}_w{ci}",
                            "opcode": "Drain",
                            "outs": [],
                            "sync_info": {"on_update": [], "on_wait": chunk},
                        }
                    )
                si = dict(si)
                si["on_wait"] = chunks[-1]
                inst = dict(inst)
                inst["sync_info"] = si
                new_insts.append(inst)
            blk["instructions"] = new_insts
    return bir


def _install_patch(nc):
    import concourse.mybir as mybir

    def to_json_bytes_patched():
        bir = json.loads(mybir.module_to_json_bytes(nc.m))
        return json.dumps(_split_waits(bir)).encode()

    nc.to_json_bytes = to_json_bytes_patched
    return nc


# --- device kernel ---------------------------------------------------------

def _build(C, repeats=1):
    """Build the per-core Bass program for token capacity C (multiple of T).

    repeats re-runs the whole token loop (same data, same outputs); it exists
    only so test harnesses can measure steady-state kernel time by slope.
    """
    import concourse.bass as bass
    import concourse.mybir as mybir
    import concourse.tile as tile

    f16, f32 = mybir.dt.float16, mybir.dt.float32
    Relu = mybir.ActivationFunctionType.Relu
    Ident = mybir.ActivationFunctionType.Identity
    ts = bass.ts

    nc = bass.Bass()
    xT = nc.dram_tensor("xT", [D, C], f16, kind="ExternalInput")
    w1 = nc.dram_tensor("w1", [D, H], f16, kind="ExternalInput")
    w2 = nc.dram_tensor("w2", [H, D], f16, kind="ExternalInput")
    b1 = nc.dram_tensor("b1", [P, HO], f32, kind="ExternalInput")
    b2 = nc.dram_tensor("b2", [P, DO], f32, kind="ExternalInput")
    yT = nc.dram_tensor("yT", [D, C], f32, kind="ExternalOutput")

    xTr = xT.rearrange("(do di) c -> di do c", di=P)
    yTr = yT.rearrange("(do di) c -> di do c", di=P)
    w1r = w1.rearrange("(do di) h -> di do h", di=P)
    w2r = w2.rearrange("(ho hi) d -> hi ho d", hi=P)
    blocks = _block_schedule(C)

    with tile.TileContext(nc) as tc:
        with (
            tc.tile_pool(name="wp", bufs=1) as wp,
            tc.tile_pool(name="xp", bufs=2) as xp,
            tc.tile_pool(name="hp", bufs=1) as hp,
            tc.tile_pool(name="yp", bufs=1) as yp,
            tc.tile_pool(name="pp", bufs=6, space=bass.MemorySpace.PSUM) as pp,
        ):
            # Block-0 input + biases first: the HWDGE queues drain in
            # program order, so anything emitted after the 16 MB of weights
            # would stall the first matmuls behind the whole weight load.
            xb0 = xp.tile([P, DO, blocks[0]], f16, tag="xb")
            nc.sync.dma_start(xb0[:], xTr[:, :, bass.ds(0, blocks[0])])
            b1_sb = wp.tile([P, HO], f32)
            nc.sync.dma_start(b1_sb[:], b1[:])
            b2_sb = wp.tile([P, DO], f32)
            nc.sync.dma_start(b2_sb[:], b2[:])
            w1_sb = wp.tile([P, DO, H], f16)
            for q in range(8):
                nc.sync.dma_start(
                    w1_sb[:, :, ts(q, H // 8)], w1r[:, :, ts(q, H // 8)]
                )
            w2_sb = wp.tile([P, HO, D], f16)
            for q in range(4):
                nc.sync.dma_start(
                    w2_sb[:, ts(q, HO // 4), :], w2r[:, ts(q, HO // 4), :]
                )

            first = True
            for _rep in range(repeats):
              off = 0
              for tb in blocks:
                sl = bass.ds(off, tb)
                if first:
                    xb = xb0
                    first = False
                else:
                    xb = xp.tile([P, DO, tb], f16, tag="xb")
                    nc.sync.dma_start(xb[:], xTr[:, :, sl])

                hb = hp.tile([P, HO, tb], f16, tag="hb")
                for ho in range(HO):
                    ps = pp.tile([P, tb], f32, tag="ps")
                    for do in range(DO):
                        nc.tensor.matmul(
                            ps[:],
                            w1_sb[:, do, ts(ho, P)],
                            xb[:, do, :],
                            start=(do == 0),
                            stop=(do == DO - 1),
                        )
                    nc.scalar.activation(
                        hb[:, ho, :], ps[:], Relu, bias=b1_sb[:, ho : ho + 1]
                    )

                yb = yp.tile([P, DO, tb], f32, tag="yb")
                for do in range(DO):
                    ps = pp.tile([P, tb], f32, tag="ps")
                    for ho in range(HO):
                        nc.tensor.matmul(
                            ps[:],
                            w2_sb[:, ho, ts(do, P)],
                            hb[:, ho, :],
                            start=(ho == 0),
                            stop=(ho == HO - 1),
                        )
                    nc.scalar.activation(
                        yb[:, do, :], ps[:], Ident, bias=b2_sb[:, do : do + 1]
                    )
                nc.sync.dma_start(yTr[:, :, sl], yb[:])
                off += tb

    return _install_patch(nc)


def build_nc(C, repeats=1):
    key = (C, repeats)
    nc = _NC_CACHE.get(key)
    if nc is None:
        nc = _NC_CACHE[key] = _build(C, repeats)
    return nc


# --- host dispatch / combine ----------------------------------------------

def make_in_maps(input_batch, indices, W1, b1, W2, b2):
    """Gather tokens by expert; returns (in_maps, order, counts, C)."""
    x = np.asarray(input_batch, dtype=np.float32)
    idx = np.asarray(indices).astype(np.int64)
    n = x.shape[0]

    flat_e = idx.reshape(-1)
    flat_tok = np.repeat(np.arange(n), TOPK)
    order = np.argsort(flat_e, kind="stable")
    counts = np.bincount(flat_e, minlength=E).astype(np.int64)
    bounds = np.concatenate([[0], np.cumsum(counts)])
    C = max(P, int(-(-counts.max() // P) * P))

    x16 = x.astype(np.float16)
    tok_order = flat_tok[order]
    W1 = np.asarray(W1)
    W2 = np.asarray(W2)
    b1 = np.asarray(b1, dtype=np.float32)
    b2 = np.asarray(b2, dtype=np.float32)

    in_maps = []
    for e in range(E):
        rows = tok_order[bounds[e] : bounds[e + 1]]
        xT_e = np.zeros((D, C), np.float16)
        xT_e[:, : len(rows)] = x16[rows].T
        in_maps.append(
            {
                "xT": xT_e,
                "w1": np.ascontiguousarray(W1[e], dtype=np.float16),
                "w2": np.ascontiguousarray(W2[e], dtype=np.float16),
                "b1": np.ascontiguousarray(b1[e].reshape(HO, P).T),
                "b2": np.ascontiguousarray(b2[e].reshape(DO, P).T),
            }
        )
    return in_maps, order, counts, bounds, C


def combine(results, probabilities, order, counts, bounds, n):
    """Scatter per-expert outputs back and apply router probabilities."""
    flat_p = np.asarray(probabilities, dtype=np.float32).reshape(-1)
    out_flat = np.zeros((n * TOPK, D), np.float32)
    for e in range(E):
        c_e = int(counts[e])
        if c_e == 0:
            continue
        seg = order[bounds[e] : bounds[e + 1]]
        ye = results[e]["yT"][:, :c_e].T  # [c_e, D]
        out_flat[seg] = ye * flat_p[seg][:, None]
    return out_flat.reshape(n, TOPK, D).sum(axis=1)


def kernel(input_batch, probabilities, indices, W1, b1, W2, b2):
    from concourse import bass_utils

    n = np.asarray(input_batch).shape[0]
    in_maps, order, counts, bounds, C = make_in_maps(
        input_batch, indices, W1, b1, W2, b2
    )
    nc = build_nc(C)
    res = bass_utils.run_bass_kernel_spmd(nc, in_maps, core_ids=list(range(E)))
    return combine(res.results, probabilities, order, counts, bounds, n)


# revision 11
# speedup vs baseline: 204.1698x; 204.1698x over previous
"""Mixture-of-Experts forward on 8 Trainium2 NeuronCores.

Strategy: expert-parallel. The host dispatches tokens to experts (gather by
router indices), each of the 8 cores runs one expert's MLP
(Linear(D->H) + ReLU + Linear(H->D)) over its token batch in fp16 with fp32
PSUM accumulation, and the host applies router probabilities and
scatter-adds the per-(token, slot) outputs back to the full [N, D] output.

Device-side layout: tokens live on the matmul moving (free) dimension and
the contraction dims (D for layer 1, H for layer 2) live on SBUF
partitions, so neither weights nor activations ever need an on-device
transpose: layer 1 computes hT = W1slice.T @ xT and layer 2 computes
yT = W2slice.T @ hT.
"""

import json

import numpy as np

N_TOK = 8192
D, H, E, TOPK, P = 1024, 4096, 8, 2, 128
T = 512  # main token tile per matmul (moving dim; <=512 fp32 PSUM bank)
DO, HO = D // P, H // P


def _block_schedule(C):
    """Split capacity C into matmul moving-dim blocks (last may be short)."""
    blocks = [T] * (C // T)
    if C % T:
        blocks.append(C % T)
    return blocks

_NC_CACHE = {}


# --- walrus workaround -----------------------------------------------------
# The installed walrus rejects control instructions (Drain) carrying more
# than a couple of semaphore waits ("Too many sync wait commands",
# CoreV3GenImpl.cpp setupSyncWait<...CTRL_NO_STRUCT>). TileContext's final
# global-clock drain collects one wait per logical processor, which trips
# this. Split long on_wait lists across several consecutive same-engine
# Drain instructions at the BIR-JSON level.

def _split_waits(bir, max_waits=1):
    for fn in bir.get("functions", []):
        for blk in fn.get("blocks", []):
            new_insts = []
            for inst in blk.get("instructions", []):
                si = inst.get("sync_info") or {}
                waits = si.get("on_wait") or []
                if len(waits) <= max_waits:
                    new_insts.append(inst)
                    continue
                chunks = [
                    waits[i : i + max_waits]
                    for i in range(0, len(waits), max_waits)
                ]
                for ci, chunk in enumerate(chunks[:-1]):
                    new_insts.append(
                        {
                            "debug": inst.get("debug"),
                            "engine": inst["engine"],
                            "ins": [],
                            "name": f"{inst['name']}_w{ci}",
                            "opcode": "Drain",
                            "outs": [],
                            "sync_info": {"on_update": [], "on_wait": chunk},
                        }
                    )
                si = dict(si)
                si["on_wait"] = chunks[-1]
                inst = dict(inst)
                inst["sync_info"] = si
                new_insts.append(inst)
            blk["instructions"] = new_insts
    return bir


def _install_patch(nc):
    import concourse.mybir as mybir

    def to_json_bytes_patched():
        bir = json.loads(mybir.module_to_json_bytes(nc.m))
        return json.dumps(_split_waits(bir)).encode()

    nc.to_json_bytes = to_json_bytes_patched
    return nc


# --- device kernel ---------------------------------------------------------

def _build(C, repeats=1, weights_inside=False):
    """Build the per-core Bass program for token capacity C (multiple of T).

    repeats re-runs the whole token loop (same data, same outputs); it exists
    only so test harnesses can measure steady-state kernel time by slope.
    weights_inside moves the weight DMAs inside the repeat loop so the slope
    also includes the weight-load warmup of a fresh run.
    """
    import concourse.bass as bass
    import concourse.mybir as mybir
    import concourse.tile as tile

    f16, f32 = mybir.dt.float16, mybir.dt.float32
    Relu = mybir.ActivationFunctionType.Relu
    Ident = mybir.ActivationFunctionType.Identity
    ts = bass.ts

    nc = bass.Bass()
    xT = nc.dram_tensor("xT", [D, C], f16, kind="ExternalInput")
    w1 = nc.dram_tensor("w1", [D, H], f16, kind="ExternalInput")
    w2 = nc.dram_tensor("w2", [H, D], f16, kind="ExternalInput")
    b1 = nc.dram_tensor("b1", [P, HO], f32, kind="ExternalInput")
    b2 = nc.dram_tensor("b2", [P, DO], f32, kind="ExternalInput")
    yT = nc.dram_tensor("yT", [D, C], f32, kind="ExternalOutput")

    xTr = xT.rearrange("(do di) c -> di do c", di=P)
    yTr = yT.rearrange("(do di) c -> di do c", di=P)
    w1r = w1.rearrange("(do di) h -> di do h", di=P)
    w2r = w2.rearrange("(ho hi) d -> hi ho d", hi=P)
    blocks = _block_schedule(C)

    with tile.TileContext(nc) as tc:
        with (
            tc.tile_pool(name="wp", bufs=1) as wp,
            tc.tile_pool(name="xp", bufs=2) as xp,
            tc.tile_pool(name="hp", bufs=1) as hp,
            tc.tile_pool(name="yp", bufs=1) as yp,
            tc.tile_pool(name="pp", bufs=6, space=bass.MemorySpace.PSUM) as pp,
        ):
            # Block-0 input + biases first: the HWDGE queues drain in
            # program order, so anything emitted after the 16 MB of weights
            # would stall the first matmuls behind the whole weight load.
            def load_weights():
                xb0 = xp.tile([P, DO, blocks[0]], f16, tag="xb")
                nc.sync.dma_start(xb0[:], xTr[:, :, bass.ds(0, blocks[0])])
                b1_sb = wp.tile([P, HO], f32, tag="b1")
                nc.sync.dma_start(b1_sb[:], b1[:])
                b2_sb = wp.tile([P, DO], f32, tag="b2")
                nc.sync.dma_start(b2_sb[:], b2[:])
                w1_sb = wp.tile([P, DO, H], f16, tag="w1")
                for q in range(8):
                    nc.sync.dma_start(
                        w1_sb[:, :, ts(q, H // 8)], w1r[:, :, ts(q, H // 8)]
                    )
                w2_sb = wp.tile([P, HO, D], f16, tag="w2")
                for q in range(4):
                    nc.sync.dma_start(
                        w2_sb[:, ts(q, HO // 4), :], w2r[:, ts(q, HO // 4), :]
                    )
                return xb0, b1_sb, b2_sb, w1_sb, w2_sb

            if not weights_inside:
                xb0, b1_sb, b2_sb, w1_sb, w2_sb = load_weights()

            first = True
            for _rep in range(repeats):
              if weights_inside:
                  xb0, b1_sb, b2_sb, w1_sb, w2_sb = load_weights()
                  first = True
              off = 0
              for tb in blocks:
                sl = bass.ds(off, tb)
                if first:
                    xb = xb0
                    first = False
                else:
                    xb = xp.tile([P, DO, tb], f16, tag="xb")
                    nc.sync.dma_start(xb[:], xTr[:, :, sl])

                hb = hp.tile([P, HO, tb], f16, tag="hb")
                for ho in range(HO):
                    ps = pp.tile([P, tb], f32, tag="ps")
                    for do in range(DO):
                        nc.tensor.matmul(
                            ps[:],
                            w1_sb[:, do, ts(ho, P)],
                            xb[:, do, :],
                            start=(do == 0),
                            stop=(do == DO - 1),
                        )
                    nc.scalar.activation(
                        hb[:, ho, :], ps[:], Relu, bias=b1_sb[:, ho : ho + 1]
                    )

                yb = yp.tile([P, DO, tb], f32, tag="yb")
                for do in range(DO):
                    ps = pp.tile([P, tb], f32, tag="ps")
                    for ho in range(HO):
                        nc.tensor.matmul(
                            ps[:],
                            w2_sb[:, ho, ts(do, P)],
                            hb[:, ho, :],
                            start=(ho == 0),
                            stop=(ho == HO - 1),
                        )
                    nc.scalar.activation(
                        yb[:, do, :], ps[:], Ident, bias=b2_sb[:, do : do + 1]
                    )
                nc.sync.dma_start(yTr[:, :, sl], yb[:])
                off += tb

    return _install_patch(nc)


def build_nc(C, repeats=1, weights_inside=False):
    key = (C, repeats, weights_inside)
    nc = _NC_CACHE.get(key)
    if nc is None:
        nc = _NC_CACHE[key] = _build(C, repeats, weights_inside)
    return nc


# --- host dispatch / combine ----------------------------------------------

def make_in_maps(input_batch, indices, W1, b1, W2, b2):
    """Gather tokens by expert; returns (in_maps, order, counts, C)."""
    x = np.asarray(input_batch, dtype=np.float32)
    idx = np.asarray(indices).astype(np.int64)
    n = x.shape[0]

    flat_e = idx.reshape(-1)
    flat_tok = np.repeat(np.arange(n), TOPK)
    order = np.argsort(flat_e, kind="stable")
    counts = np.bincount(flat_e, minlength=E).astype(np.int64)
    bounds = np.concatenate([[0], np.cumsum(counts)])
    # capacity = max expert load, rounded up a little for DMA-friendly
    # alignment; the matmul moving dim has no 128-multiple requirement
    C = max(P, int(-(-counts.max() // 8) * 8))

    x16 = x.astype(np.float16)
    tok_order = flat_tok[order]
    W1 = np.asarray(W1)
    W2 = np.asarray(W2)
    b1 = np.asarray(b1, dtype=np.float32)
    b2 = np.asarray(b2, dtype=np.float32)

    in_maps = []
    for e in range(E):
        rows = tok_order[bounds[e] : bounds[e + 1]]
        xT_e = np.zeros((D, C), np.float16)
        xT_e[:, : len(rows)] = x16[rows].T
        in_maps.append(
            {
                "xT": xT_e,
                "w1": np.ascontiguousarray(W1[e], dtype=np.float16),
                "w2": np.ascontiguousarray(W2[e], dtype=np.float16),
                "b1": np.ascontiguousarray(b1[e].reshape(HO, P).T),
                "b2": np.ascontiguousarray(b2[e].reshape(DO, P).T),
            }
        )
    return in_maps, order, counts, bounds, C


def combine(results, probabilities, order, counts, bounds, n):
    """Scatter per-expert outputs back and apply router probabilities."""
    flat_p = np.asarray(probabilities, dtype=np.float32).reshape(-1)
    out_flat = np.zeros((n * TOPK, D), np.float32)
    for e in range(E):
        c_e = int(counts[e])
        if c_e == 0:
            continue
        seg = order[bounds[e] : bounds[e + 1]]
        ye = results[e]["yT"][:, :c_e].T  # [c_e, D]
        out_flat[seg] = ye * flat_p[seg][:, None]
    return out_flat.reshape(n, TOPK, D).sum(axis=1)


_RUNNER_CACHE = {}


def _make_runner(nc, n_cores):
    """Build a reusable jitted SPMD executor for nc (axon/PJRT path only).

    Mirrors bass2jax.run_bass_via_pjrt but caches the jitted function so
    repeat kernel() calls skip re-tracing/re-compiling.
    """
    import jax
    import concourse.mybir as mybir
    from concourse import bass2jax
    from jax.sharding import Mesh, NamedSharding, PartitionSpec


    bass2jax.install_neuronx_cc_hook()
    partition_name = (
        nc.partition_id_tensor.name if nc.partition_id_tensor else None
    )
    in_names, out_names, out_avals, out_shapes = [], [], [], []
    for alloc in nc.m.functions[0].allocations:
        if not isinstance(alloc, mybir.MemoryLocationSet):
            continue
        name = alloc.memorylocations[0].name
        if alloc.kind == "ExternalInput":
            if name != partition_name:
                in_names.append(name)
        elif alloc.kind == "ExternalOutput":
            shape = tuple(alloc.tensor_shape)
            dtype = mybir.dt.np(alloc.dtype)
            out_names.append(name)
            out_avals.append(jax.core.ShapedArray(shape, dtype))
            out_shapes.append((shape, dtype))
    n_params = len(in_names)
    all_in_names = in_names + out_names + (
        [partition_name] if partition_name else []
    )

    def _body(*args):
        operands = list(args)
        if partition_name is not None:
            operands.append(bass2jax.partition_id_tensor())
        return tuple(
            bass2jax._bass_exec_p.bind(
                *operands,
                out_avals=tuple(out_avals),
                in_names=tuple(all_in_names),
                out_names=tuple(out_names),
                lowering_input_output_aliases=(),
                sim_require_finite=True,
                sim_require_nnan=True,
                nc=nc,
            )
        )

    devices = jax.devices()[:n_cores]
    mesh = Mesh(np.asarray(devices), ("core",))
    n_outs = len(out_names)
    try:
        from jax.experimental.shard_map import shard_map

        smapped = shard_map(
            _body,
            mesh=mesh,
            in_specs=(PartitionSpec("core"),) * (n_params + n_outs),
            out_specs=(PartitionSpec("core"),) * n_outs,
            check_rep=False,
        )
    except (ImportError, TypeError):
        from jax import shard_map

        smapped = shard_map(
            _body,
            mesh=mesh,
            in_specs=(PartitionSpec("core"),) * (n_params + n_outs),
            out_specs=(PartitionSpec("core"),) * n_outs,
            check_vma=False,
        )
    fn = jax.jit(smapped, keep_unused=True)
    sharding = NamedSharding(mesh, PartitionSpec("core"))

    def run(in_maps):
        concat_in = [
            np.concatenate([in_maps[c][nm] for c in range(n_cores)], axis=0)
            for nm in in_names
        ]
        concat_zeros = [
            np.zeros((n_cores * s[0], *s[1:]), dt) for s, dt in out_shapes
        ]
        dev_in = [jax.device_put(a, sharding) for a in concat_in]
        dev_zero = [jax.device_put(a, sharding) for a in concat_zeros]
        out = fn(*dev_in, *dev_zero)
        return [
            {
                nm: np.asarray(out[i]).reshape(n_cores, *out_shapes[i][0])[c]
                for i, nm in enumerate(out_names)
            }
            for c in range(n_cores)
        ]

    return run


def _run_spmd(nc, in_maps, C):
    """Run the program on cores 0..E-1; cache the executable per capacity."""
    from concourse._compat import axon_active

    if axon_active():
        try:
            runner = _RUNNER_CACHE.get(C)
            if runner is None:
                runner = _RUNNER_CACHE[C] = _make_runner(nc, len(in_maps))
            return runner(in_maps)
        except Exception:
            _RUNNER_CACHE.pop(C, None)
    from concourse import bass_utils

    res = bass_utils.run_bass_kernel_spmd(
        nc, in_maps, core_ids=list(range(len(in_maps)))
    )
    return res.results


def kernel(input_batch, probabilities, indices, W1, b1, W2, b2):
    n = np.asarray(input_batch).shape[0]
    in_maps, order, counts, bounds, C = make_in_maps(
        input_batch, indices, W1, b1, W2, b2
    )
    nc = build_nc(C)
    results = _run_spmd(nc, in_maps, C)
    return combine(results, probabilities, order, counts, bounds, n)


# revision 15
# speedup vs baseline: 239.7427x; 1.1742x over previous
"""Mixture-of-Experts forward on 8 Trainium2 NeuronCores.

Strategy: expert-parallel. The host dispatches tokens to experts (gather by
router indices), each of the 8 cores runs one expert's MLP
(Linear(D->H) + ReLU + Linear(H->D)) over its token batch in fp16 with fp32
PSUM accumulation, and the host applies router probabilities and
scatter-adds the per-(token, slot) outputs back to the full [N, D] output.

Device-side layout: tokens live on the matmul moving (free) dimension and
the contraction dims (D for layer 1, H for layer 2) live on SBUF
partitions, so neither weights nor activations ever need an on-device
transpose: layer 1 computes hT = W1slice.T @ xT and layer 2 computes
yT = W2slice.T @ hT.
"""

import json

import numpy as np

N_TOK = 8192
D, H, E, TOPK, P = 1024, 4096, 8, 2, 128
T = 512  # main token tile per matmul (moving dim; <=512 fp32 PSUM bank)
DO, HO = D // P, H // P


def _block_schedule(C):
    """Split capacity C into near-equal moving-dim blocks of at most T.

    Equal widths (instead of T-sized blocks plus a narrow remainder) keep
    every matmul's moving dim wide enough that LDWEIGHTS stays hidden under
    the rhs stream; a narrow tail block would be weight-load-bound.
    """
    nb = -(-C // T)
    u = C // 2  # schedule in units of 2 tokens so block widths stay even
    q, r = divmod(u, nb)
    return [2 * (q + (1 if i < r else 0)) for i in range(nb)]

_NC_CACHE = {}


# --- walrus workaround -----------------------------------------------------
# The installed walrus rejects control instructions (Drain) carrying more
# than a couple of semaphore waits ("Too many sync wait commands",
# CoreV3GenImpl.cpp setupSyncWait<...CTRL_NO_STRUCT>). TileContext's final
# global-clock drain collects one wait per logical processor, which trips
# this. Split long on_wait lists across several consecutive same-engine
# Drain instructions at the BIR-JSON level.

def _split_waits(bir, max_waits=1):
    for fn in bir.get("functions", []):
        for blk in fn.get("blocks", []):
            new_insts = []
            for inst in blk.get("instructions", []):
                si = inst.get("sync_info") or {}
                waits = si.get("on_wait") or []
                if len(waits) <= max_waits:
                    new_insts.append(inst)
                    continue
                chunks = [
                    waits[i : i + max_waits]
                    for i in range(0, len(waits), max_waits)
                ]
                for ci, chunk in enumerate(chunks[:-1]):
                    new_insts.append(
                        {
                            "debug": inst.get("debug"),
                            "engine": inst["engine"],
                            "ins": [],
                            "name": f"{inst['name']}_w{ci}",
                            "opcode": "Drain",
                            "outs": [],
                            "sync_info": {"on_update": [], "on_wait": chunk},
                        }
                    )
                si = dict(si)
                si["on_wait"] = chunks[-1]
                inst = dict(inst)
                inst["sync_info"] = si
                new_insts.append(inst)
            blk["instructions"] = new_insts
    return bir


def _install_patch(nc):
    import concourse.mybir as mybir

    def to_json_bytes_patched():
        bir = json.loads(mybir.module_to_json_bytes(nc.m))
        return json.dumps(_split_waits(bir)).encode()

    nc.to_json_bytes = to_json_bytes_patched
    return nc


# --- device kernel ---------------------------------------------------------

def _build(C, repeats=1, weights_inside=False):
    """Build the per-core Bass program for token capacity C.

    repeats re-runs the whole token loop (same data, same outputs); it exists
    only so test harnesses can measure steady-state kernel time by slope.
    weights_inside moves the weight DMAs inside the repeat loop so the slope
    also includes the weight-load warmup of a fresh run.
    """
    import concourse.bass as bass
    import concourse.mybir as mybir
    import concourse.tile as tile

    f16, f32 = mybir.dt.float16, mybir.dt.float32
    Relu = mybir.ActivationFunctionType.Relu
    Ident = mybir.ActivationFunctionType.Identity
    ts = bass.ts

    nc = bass.Bass()
    xT = nc.dram_tensor("xT", [D, C], f16, kind="ExternalInput")
    w1 = nc.dram_tensor("w1", [D, H], f16, kind="ExternalInput")
    w2 = nc.dram_tensor("w2", [H, D], f16, kind="ExternalInput")
    b1 = nc.dram_tensor("b1", [P, HO], f32, kind="ExternalInput")
    b2 = nc.dram_tensor("b2", [P, DO], f32, kind="ExternalInput")
    yT = nc.dram_tensor("yT", [D, C], f32, kind="ExternalOutput")

    xTr = xT.rearrange("(do di) c -> di do c", di=P)
    yTr = yT.rearrange("(do di) c -> di do c", di=P)
    w1r = w1.rearrange("(do di) h -> di do h", di=P)
    w2r = w2.rearrange("(ho hi) d -> hi ho d", hi=P)
    blocks = _block_schedule(C)

    with tile.TileContext(nc) as tc:
        with (
            tc.tile_pool(name="wp", bufs=1) as wp,
            tc.tile_pool(name="xp", bufs=2) as xp,
            tc.tile_pool(name="hp", bufs=1) as hp,
            tc.tile_pool(name="yp", bufs=1) as yp,
            tc.tile_pool(name="pp", bufs=8, space=bass.MemorySpace.PSUM) as pp,
        ):
            # Block-0 input + biases first: the HWDGE queues drain in
            # program order, so anything emitted after the 16 MB of weights
            # would stall the first matmuls behind the whole weight load.
            def load_weights():
                xb0 = xp.tile([P, DO, blocks[0]], f16, tag="xb")
                nc.sync.dma_start(xb0[:], xTr[:, :, bass.ds(0, blocks[0])])
                b1_sb = wp.tile([P, HO], f32, tag="b1")
                nc.sync.dma_start(b1_sb[:], b1[:])
                b2_sb = wp.tile([P, DO], f32, tag="b2")
                nc.sync.dma_start(b2_sb[:], b2[:])
                w1_sb = wp.tile([P, DO, H], f16, tag="w1")
                for q in range(16):
                    nc.sync.dma_start(
                        w1_sb[:, :, ts(q, H // 16)], w1r[:, :, ts(q, H // 16)]
                    )
                w2_sb = wp.tile([P, HO, D], f16, tag="w2")
                for q in range(8):
                    nc.sync.dma_start(
                        w2_sb[:, ts(q, HO // 8), :], w2r[:, ts(q, HO // 8), :]
                    )
                return xb0, b1_sb, b2_sb, w1_sb, w2_sb

            if not weights_inside:
                xb0, b1_sb, b2_sb, w1_sb, w2_sb = load_weights()

            first = True
            for _rep in range(repeats):
              if weights_inside:
                  xb0, b1_sb, b2_sb, w1_sb, w2_sb = load_weights()
                  first = True
              off = 0
              for tb in blocks:
                sl = bass.ds(off, tb)
                if first:
                    xb = xb0
                    first = False
                else:
                    xb = xp.tile([P, DO, tb], f16, tag="xb")
                    nc.sync.dma_start(xb[:], xTr[:, :, sl])

                hb = hp.tile([P, HO, tb], f16, tag="hb")
                for ho in range(HO):
                    ps = pp.tile([P, tb], f32, tag="ps")
                    for do in range(DO):
                        nc.tensor.matmul(
                            ps[:],
                            w1_sb[:, do, ts(ho, P)],
                            xb[:, do, :],
                            start=(do == 0),
                            stop=(do == DO - 1),
                        )
                    nc.scalar.activation(
                        hb[:, ho, :], ps[:], Relu, bias=b1_sb[:, ho : ho + 1]
                    )

                yb = yp.tile([P, DO, tb], f32, tag="yb")
                for do in range(DO):
                    ps = pp.tile([P, tb], f32, tag="ps")
                    for ho in range(HO):
                        nc.tensor.matmul(
                            ps[:],
                            w2_sb[:, ho, ts(do, P)],
                            hb[:, ho, :],
                            start=(ho == 0),
                            stop=(ho == HO - 1),
                        )
                    nc.scalar.activation(
                        yb[:, do, :], ps[:], Ident, bias=b2_sb[:, do : do + 1]
                    )
                nc.sync.dma_start(yTr[:, :, sl], yb[:])
                off += tb

    return _install_patch(nc)


def build_nc(C, repeats=1, weights_inside=False):
    key = (C, repeats, weights_inside)
    nc = _NC_CACHE.get(key)
    if nc is None:
        nc = _NC_CACHE[key] = _build(C, repeats, weights_inside)
    return nc


# --- host dispatch / combine ----------------------------------------------

def make_in_maps(input_batch, indices, W1, b1, W2, b2):
    """Gather tokens by expert; returns (in_maps, order, counts, bounds, C)."""
    x = np.asarray(input_batch, dtype=np.float32)
    idx = np.asarray(indices).astype(np.int64)
    n = x.shape[0]

    flat_e = idx.reshape(-1)
    flat_tok = np.repeat(np.arange(n), TOPK)
    order = np.argsort(flat_e, kind="stable")
    counts = np.bincount(flat_e, minlength=E).astype(np.int64)
    bounds = np.concatenate([[0], np.cumsum(counts)])
    # capacity = max expert load, rounded up a little for DMA-friendly
    # alignment; the matmul moving dim has no 128-multiple requirement
    C = max(P, int(-(-counts.max() // 8) * 8))

    x16 = x.astype(np.float16)
    tok_order = flat_tok[order]
    W1 = np.asarray(W1)
    W2 = np.asarray(W2)
    b1 = np.asarray(b1, dtype=np.float32)
    b2 = np.asarray(b2, dtype=np.float32)

    in_maps = []
    for e in range(E):
        rows = tok_order[bounds[e] : bounds[e + 1]]
        xT_e = np.zeros((D, C), np.float16)
        xT_e[:, : len(rows)] = x16[rows].T
        in_maps.append(
            {
                "xT": xT_e,
                "w1": np.ascontiguousarray(W1[e], dtype=np.float16),
                "w2": np.ascontiguousarray(W2[e], dtype=np.float16),
                "b1": np.ascontiguousarray(b1[e].reshape(HO, P).T),
                "b2": np.ascontiguousarray(b2[e].reshape(DO, P).T),
            }
        )
    return in_maps, order, counts, bounds, C


def combine(results, probabilities, order, counts, bounds, n):
    """Scatter per-expert outputs back and apply router probabilities."""
    flat_p = np.asarray(probabilities, dtype=np.float32).reshape(-1)
    out_flat = np.zeros((n * TOPK, D), np.float32)
    for e in range(E):
        c_e = int(counts[e])
        if c_e == 0:
            continue
        seg = order[bounds[e] : bounds[e + 1]]
        ye = results[e]["yT"][:, :c_e].T  # [c_e, D]
        out_flat[seg] = ye * flat_p[seg][:, None]
    return out_flat.reshape(n, TOPK, D).sum(axis=1)


_RUNNER_CACHE = {}


def _make_runner_parts(nc, n_cores):
    """Build a reusable jitted SPMD executor for nc (axon/PJRT path only).

    Mirrors bass2jax.run_bass_via_pjrt but caches the jitted function so
    repeat kernel() calls skip re-tracing/re-compiling. Returns
    (fn, put, unpack): put(in_maps) -> device args, fn(*args) -> device outs,
    unpack(outs) -> per-core result dicts.
    """
    import jax
    import concourse.mybir as mybir
    from concourse import bass2jax
    from jax.sharding import Mesh, NamedSharding, PartitionSpec


    bass2jax.install_neuronx_cc_hook()
    partition_name = (
        nc.partition_id_tensor.name if nc.partition_id_tensor else None
    )
    in_names, out_names, out_avals, out_shapes = [], [], [], []
    for alloc in nc.m.functions[0].allocations:
        if not isinstance(alloc, mybir.MemoryLocationSet):
            continue
        name = alloc.memorylocations[0].name
        if alloc.kind == "ExternalInput":
            if name != partition_name:
                in_names.append(name)
        elif alloc.kind == "ExternalOutput":
            shape = tuple(alloc.tensor_shape)
            dtype = mybir.dt.np(alloc.dtype)
            out_names.append(name)
            out_avals.append(jax.core.ShapedArray(shape, dtype))
            out_shapes.append((shape, dtype))
    n_params = len(in_names)
    all_in_names = in_names + out_names + (
        [partition_name] if partition_name else []
    )

    def _body(*args):
        operands = list(args)
        if partition_name is not None:
            operands.append(bass2jax.partition_id_tensor())
        return tuple(
            bass2jax._bass_exec_p.bind(
                *operands,
                out_avals=tuple(out_avals),
                in_names=tuple(all_in_names),
                out_names=tuple(out_names),
                lowering_input_output_aliases=(),
                sim_require_finite=True,
                sim_require_nnan=True,
                nc=nc,
            )
        )

    devices = jax.devices()[:n_cores]
    mesh = Mesh(np.asarray(devices), ("core",))
    n_outs = len(out_names)
    try:
        from jax.experimental.shard_map import shard_map

        smapped = shard_map(
            _body,
            mesh=mesh,
            in_specs=(PartitionSpec("core"),) * (n_params + n_outs),
            out_specs=(PartitionSpec("core"),) * n_outs,
            check_rep=False,
        )
    except (ImportError, TypeError):
        from jax import shard_map

        smapped = shard_map(
            _body,
            mesh=mesh,
            in_specs=(PartitionSpec("core"),) * (n_params + n_outs),
            out_specs=(PartitionSpec("core"),) * n_outs,
            check_vma=False,
        )
    fn = jax.jit(smapped, keep_unused=True)
    sharding = NamedSharding(mesh, PartitionSpec("core"))

    def put(in_maps):
        concat_in = [
            np.concatenate([in_maps[c][nm] for c in range(n_cores)], axis=0)
            for nm in in_names
        ]
        concat_zeros = [
            np.zeros((n_cores * s[0], *s[1:]), dt) for s, dt in out_shapes
        ]
        dev_in = [jax.device_put(a, sharding) for a in concat_in]
        dev_zero = [jax.device_put(a, sharding) for a in concat_zeros]
        return dev_in + dev_zero

    def unpack(out):
        return [
            {
                nm: np.asarray(out[i]).reshape(n_cores, *out_shapes[i][0])[c]
                for i, nm in enumerate(out_names)
            }
            for c in range(n_cores)
        ]

    return fn, put, unpack


def _make_runner(nc, n_cores):
    fn, put, unpack = _make_runner_parts(nc, n_cores)

    def run(in_maps):
        return unpack(fn(*put(in_maps)))

    return run


def _run_spmd(nc, in_maps, C):
    """Run the program on cores 0..E-1; cache the executable per capacity."""
    from concourse._compat import axon_active

    if axon_active():
        try:
            runner = _RUNNER_CACHE.get(C)
            if runner is None:
                runner = _RUNNER_CACHE[C] = _make_runner(nc, len(in_maps))
            return runner(in_maps)
        except Exception:
            _RUNNER_CACHE.pop(C, None)
    from concourse import bass_utils

    res = bass_utils.run_bass_kernel_spmd(
        nc, in_maps, core_ids=list(range(len(in_maps)))
    )
    return res.results


def kernel(input_batch, probabilities, indices, W1, b1, W2, b2):
    n = np.asarray(input_batch).shape[0]
    in_maps, order, counts, bounds, C = make_in_maps(
        input_batch, indices, W1, b1, W2, b2
    )
    nc = build_nc(C)
    results = _run_spmd(nc, in_maps, C)
    return combine(results, probabilities, order, counts, bounds, n)

